# revision 17
# baseline (speedup 1.0000x reference)
"""Trainium2 Bass kernel: multi-head self-attention block (dense transformer).

Computes y = softmax((x @ Wq) (x @ Wk)^T / sqrt(H)) (x @ Wv) @ Wout + bias
for B=2, T=2048, C=1024, H=16 heads of dim 64, fp32 I/O.

Sharding: tensor-parallel over heads. Each of the 8 NeuronCores computes the
QKV projections, attention, and output projection for 2 of the 16 heads
(both batches), producing a partial output y_c = attn_out_c @ Wout[rows_c].
The host gather sums the 8 partials and adds the output bias.

v2 (this file): fp8 DoubleRow matmuls for QKV and attn@V, early attention
start, and a leaner normalize path.

  - QKV projections run as fp8e4 DoubleRow matmuls: x^T and W are cast to
    fp8 on the host (W pre-scaled by 64 to clear the fp8 subnormal range;
    V's W by 32) and e-tile PAIRS are contracted per matmul (virtual K=256),
    halving the matmul count. The 64*64 score scale surplus is folded into
    the exp scale; V's 32x rides through the softmax denominator (the
    appended ones-column is set to 32 so it cancels exactly).
  - scores^T[k,q] = (K^T tile).T @ Q^T stay bf16 (stream-bound; fp8 wins
    nothing): two heads' K=64 matmuls go to disjoint PE row groups and
    stream concurrently; exp on the scalar engine reads the fp32 PSUM
    scores of both heads in one [128,1024] ACTIVATE and writes fp8e4
    directly into the k-tile-PAIR interleaved buffer exc2[:, kt%2, :].
  - attn@V: one fp8 DoubleRow matmul per k-tile PAIR per head
    (lhsT = VS2[:, pair, :, head] with the 32-valued denominator column,
    rhs = exc2[:, :, head]) accumulating U[d|32*sum, q] in fp32 PSUM.
  - normalize: 1/denominator via DVE fast reciprocal (read from a
    partition-0 copy), partition-broadcast by a DRAM bounce DMA, and one
    DVE multiply straight out of U's PSUM into the bf16 attn_out tile
    (no separate U->SBUF copy).
  - batch-0 head: only the K chains + Q chunk-0 chain gate the first exp;
    V chains, V transposes, remaining Q chains, batch-1 QKV, and the
    projections are all demand-pulled PE filler tasks inside the
    ACT-paced attention loops.
"""

import math
import sys
from collections import deque
from contextlib import ExitStack

for _p in ("/opt/trn_rl_repo",):
    if _p not in sys.path:
        sys.path.insert(0, _p)

import ml_dtypes
import numpy as np

import concourse.bass as bass
import concourse.tile as tile
from concourse import bacc, mybir
from concourse.masks import make_identity
from concourse.bass_utils import run_bass_kernel_spmd

B, T, C = 2, 2048, 1024
H, D = 16, 64
NCORES = 8
HPC = H // NCORES            # heads per core = 2
FPC = HPC * D                # per-core q/k/v feature slice = 128
TQ = 512                     # PSUM fp32 free-dim tile
NQC = T // TQ                # 4 q-chunks
NKT = T // 128               # 16 k-tiles
NKP = NKT // 2               # 8 k-tile pairs
NET = C // 128               # 8 embedding tiles
NEP = NET // 2               # 4 e-tile pairs
SCALE = 1.0 / math.sqrt(H)   # NOTE: reference scales by 1/sqrt(n_head)

F32 = mybir.dt.float32
BF16 = mybir.dt.bfloat16
FP8 = mybir.dt.float8e4
NPBF16 = ml_dtypes.bfloat16
NPFP8 = ml_dtypes.float8_e4m3
AF = mybir.ActivationFunctionType
DR = mybir.MatmulPerfMode.DoubleRow

USE_FP8 = False              # fp8 DoubleRow QKV + attn@V (too lossy: ~5% RMS; budget ~2%)
WQ_SCALE = 64.0              # host pre-scale for Wq/Wk (fp8 subnormal fix)
WV_SCALE = 32.0              # host pre-scale for Wv; cancelled via denom
# VS2 free-dim layout: [head0 d0..63, den0, pad..] [head1 d0..63, den1, pad..]
VSW = 80                     # per-head stride in VS2 (16B-aligned for fp8)
EXP_SHIFT = 2.5              # exp(s*scale - shift): cancels in softmax


def build_nc():
    nc = bacc.Bacc(None, target_bir_lowering=False)

    if USE_FP8:
        xT = nc.declare_dram_parameter("xT", [B, C, T], FP8, isOutput=False)
        wq = nc.declare_dram_parameter("wq", [128, NET, FPC], FP8, isOutput=False)
        wk = nc.declare_dram_parameter("wk", [128, NET, FPC], FP8, isOutput=False)
        wv = nc.declare_dram_parameter("wv", [128, NET, FPC], FP8, isOutput=False)
    else:
        xT = nc.declare_dram_parameter("xT", [B, C, T], BF16, isOutput=False)
        wq = nc.declare_dram_parameter("wq", [128, NET, FPC], BF16, isOutput=False)
        wk = nc.declare_dram_parameter("wk", [128, NET, FPC], BF16, isOutput=False)
        wv = nc.declare_dram_parameter("wv", [128, NET, FPC], BF16, isOutput=False)
    bq = nc.declare_dram_parameter("bq", [FPC, 1], F32, isOutput=False)
    bk = nc.declare_dram_parameter("bk", [FPC, 1], F32, isOutput=False)
    bv = nc.declare_dram_parameter("bv", [FPC, 1], F32, isOutput=False)
    wo = nc.declare_dram_parameter("wo", [FPC, C], BF16, isOutput=False)
    y = nc.declare_dram_parameter("y", [B, T, C], F32, isOutput=True)

    exp_scale = SCALE / (WQ_SCALE * WQ_SCALE) if USE_FP8 else SCALE
    ones_val = WV_SCALE if USE_FP8 else 1.0
    vs_dt = FP8 if USE_FP8 else BF16

    with ExitStack() as ctx:
        tc = ctx.enter_context(tile.TileContext(nc))
        consts = ctx.enter_context(tc.tile_pool(name="consts", bufs=1))
        xtp = ctx.enter_context(tc.tile_pool(name="xtp", bufs=8))
        qkvp = ctx.enter_context(tc.tile_pool(name="qkvp", bufs=6))
        vsp = ctx.enter_context(tc.tile_pool(name="vsp", bufs=2))
        expp = ctx.enter_context(tc.tile_pool(name="expp", bufs=6))
        aop = ctx.enter_context(tc.tile_pool(name="aop", bufs=2))
        outp = ctx.enter_context(tc.tile_pool(name="outp", bufs=4))
        smallp = ctx.enter_context(tc.tile_pool(name="smallp", bufs=6))
        psum = ctx.enter_context(tc.tile_pool(name="psum", bufs=2, space="PSUM"))
        dramp = ctx.enter_context(tc.tile_pool(name="dramp", bufs=8, space="DRAM"))

        ident = consts.tile([128, 128], BF16)
        make_identity(nc, ident)
        # ~4us of dummy matmuls while the first DMAs land: flips the PE's
        # HAM clock-gate to 8/8 so the real chains start at 2.4 GHz
        warm = [psum.tile([128, 128], F32, tag="sm", bufs=2, name="warm")
                for _ in range(2)]
        for i in range(32):
            nc.tensor.matmul(warm[i % 2], lhsT=ident, rhs=ident,
                             start=True, stop=True)
        ones_row = consts.tile([1, D], F32)
        nc.vector.memset(ones_row, 1.0)
        shift_col = consts.tile([128, 1], F32)
        nc.vector.memset(shift_col, -EXP_SHIFT)

        # ---- weights (wk + x0 first: they gate the first K chain) ----
        wt_k = consts.tile([128, NET, FPC], vs_dt)
        nc.sync.dma_start(out=wt_k, in_=wk[:, :, :])
        wt_q = consts.tile([128, NET, FPC], vs_dt)
        wt_v = consts.tile([128, NET, FPC], vs_dt)
        wt_fi = (wt_q, wt_k, wt_v)
        bias_ts = []
        wo_t = consts.tile([128, C], BF16)

        # ---- x tiles: one per (batch, e-pair, q-chunk), pair-interleaved ----
        xts = {}

        def load_x(b):
            src = xT[b].rearrange("(e ki) t -> ki e t", ki=128)
            for p in range(NEP):
                xt = xtp.tile([128, 2, T], vs_dt, tag="xt", bufs=8,
                              name=f"xt{b}_{p}")
                eng = nc.sync if p % 2 == 0 else nc.gpsimd
                eng.dma_start(out=xt, in_=src[:, 2 * p:2 * p + 2, :])
                xts[(b, p)] = xt

        def qkv_tiles(b):
            QT = qkvp.tile([128, T], BF16, tag="qk", name=f"QT{b}")
            KT = qkvp.tile([128, T], BF16, tag="qk", name=f"KT{b}")
            VT = qkvp.tile([128, T], BF16, tag="qk", name=f"VT{b}")
            VS = vsp.tile([128, NKT, 2 * VSW], vs_dt, tag="vs", name=f"VS{b}")
            # denominator columns (value ones_val so V's host pre-scale
            # cancels); pad columns included so no byte stays uninitialized
            nc.vector.memset(VS[:, :, D:VSW], ones_val)
            nc.vector.memset(VS[:, :, VSW + D:2 * VSW], ones_val)
            return QT, KT, VT, VS

        def qkv_chain(b, fi, dst, qc):
            """One projection chain: q-chunk qc of Q/K/V (fi=0/1/2)."""
            ps = psum.tile([128, TQ], F32, tag="sm", bufs=2, name="qkvps")
            if USE_FP8:
                for p in range(NEP):
                    nc.tensor.matmul(
                        ps,
                        lhsT=wt_fi[fi][:, 2 * p:2 * p + 2, :],
                        rhs=xts[(b, p)][:, :, qc * TQ:(qc + 1) * TQ],
                        start=(p == 0),
                        stop=(p == NEP - 1),
                        perf_mode=DR,
                    )
            else:
                for p in range(NEP):
                    for ko in range(2):
                        nc.tensor.matmul(
                            ps,
                            lhsT=wt_fi[fi][:, 2 * p + ko, :],
                            rhs=xts[(b, p)][:, ko, qc * TQ:(qc + 1) * TQ],
                            start=(p == 0 and ko == 0),
                            stop=(p == NEP - 1 and ko == 1),
                        )
            nc.vector.tensor_scalar_add(
                out=dst[:, qc * TQ:(qc + 1) * TQ], in0=ps,
                scalar1=bias_ts[fi][:, 0:1],
            )

        def vtr_task(b, VT, VS, kt):
            """Transpose one k-tile of V^T into the pair-interleaved VS.
            (The DMA-crossbar alternative clogs the serial Sync queue at
            ~1.2us per transpose; the PE does this in ~0.2us.)"""
            pt = psum.tile([128, 128], BF16, tag="sm", bufs=2, name="vtps")
            nc.tensor.transpose(pt, VT[:, kt * 128:(kt + 1) * 128], ident)
            out_ap = VS[:, kt, :].rearrange("pp (h x) -> pp h x", h=2)[:, :, 0:D]
            in_ap = pt.rearrange("pp (h d) -> pp h d", h=2)
            nc.vector.tensor_copy(out=out_ap, in_=in_ap)

        def proj_fillers(b, AO, tts):
            tasks = []
            for tt in tts:
                def pj(tt=tt):
                    ot = outp.tile([128, C], F32, tag="out", name="ot")
                    for cc in range(C // TQ):
                        pp = psum.tile([128, TQ], F32, tag="sm", bufs=2, name="projps")
                        nc.tensor.matmul(
                            pp,
                            lhsT=AO[:, tt * 128:(tt + 1) * 128],
                            rhs=wo_t[:, cc * TQ:(cc + 1) * TQ],
                            start=True,
                            stop=True,
                        )
                        nc.vector.tensor_copy(out=ot[:, cc * TQ:(cc + 1) * TQ], in_=pp)
                    nc.gpsimd.dma_start(out=y[b, tt * 128:(tt + 1) * 128, :], in_=ot)
                tasks.append(pj)
            return tasks

        def attention(b, QT, KT, VS, fill_q, done, self_proj=False,
                      fast_tail=False, first_lag=2, lag_rest=1):
            """ACT-bound attention for batch b; pops PE filler tasks from
            fill_q each k-tile to keep the TensorEngine saturated. Emission
            order must respect data deps (Tile tracks only already-emitted
            producers), so consumers force-drain fill_q past their
            producers via the shared `done` tag set."""
            AO = aop.tile([128, T], BF16, tag="ao", name=f"AO{b}")
            n_iter = NQC * NKT
            it = 0

            def pop_fillers():
                remaining = n_iter - it
                if remaining <= 0 or not fill_q:
                    return
                k = -(-len(fill_q) // remaining)  # ceil
                for _ in range(min(k, len(fill_q))):
                    fill_q.popleft()()

            def drain_until(tag):
                while tag not in done and fill_q:
                    fill_q.popleft()()

            sc_q = deque()

            def emit_scores(g):
                qc2, kt2 = divmod(g, NKT)
                if kt2 == 0:
                    drain_until(("Q", b, qc2))
                drain_until(("K", b, kt2 // 4))
                # both heads' scores land in one 2-bank PSUM tile via
                # disjoint PE row groups; one exp covers both heads.
                ssc = psum.tile([128, 2 * TQ], F32, tag="ss", bufs=2,
                                name="ssc")
                for h in range(HPC):
                    nc.tensor.matmul(
                        ssc[:, h * TQ:(h + 1) * TQ],
                        lhsT=KT[h * D:(h + 1) * D, kt2 * 128:(kt2 + 1) * 128],
                        rhs=QT[h * D:(h + 1) * D, qc2 * TQ:(qc2 + 1) * TQ],
                        start=True,
                        stop=True,
                    )
                sc_q.append(ssc)

            def normalize(qc):
                U = Us.pop(qc)
                last = fast_tail and qc == NQC - 1
                for h in range(HPC):
                    # denominator (times ones_val) lives in U row D; the DVE
                    # fast reciprocal needs a partition-0 copy.
                    scp = smallp.tile([1, TQ], F32, tag="scp", name="scp")
                    nc.vector.tensor_copy(out=scp, in_=U[h][D:D + 1, :])
                    rec = smallp.tile([1, TQ], F32, tag="rec", name="rec")
                    with nc.allow_low_precision(reason="softmax denom"):
                        nc.vector.reciprocal_approx_fast(out=rec, in_=scp)
                    rb = smallp.tile([D, TQ], F32, tag="rb", name="rb")
                    if last:
                        # latency-critical final chunk: broadcast 1/sum on the
                        # PE (ones[1,64].T @ rec) instead of the DRAM bounce
                        bcp = psum.tile([D, TQ], F32, tag="sm", bufs=2, name="bcp")
                        nc.tensor.matmul(bcp, lhsT=ones_row, rhs=rec,
                                         start=True, stop=True)
                        nc.vector.tensor_copy(out=rb, in_=bcp)
                    else:
                        nt = dramp.tile([1, TQ], F32, tag="nrm", name="nt")
                        nc.sync.dma_start(out=nt, in_=rec)
                        nc.sync.dma_start(out=rb, in_=nt.partition_broadcast(D))
                    # multiply straight out of U's PSUM; ones_val cancels
                    # V's host pre-scale (U = s*V_sum, denom = s*w_sum).
                    nc.vector.tensor_mul(
                        out=AO[h * D:(h + 1) * D, qc * TQ:(qc + 1) * TQ],
                        in0=U[h][0:D, :],
                        in1=rb,
                    )
                if self_proj:
                    tts = range(qc * (TQ // 128), (qc + 1) * (TQ // 128))
                    fill_q.extend(proj_fillers(b, AO, tts))

            emit_scores(0)
            Us = {}
            pending = deque()   # (qc, pair, attn@V task) across qc boundaries

            def pop_pending():
                pqc, pp_, task = pending.popleft()
                drain_until(("vtr", b, 2 * pp_ + 1))
                task()
                if pp_ == NKP - 1:
                    normalize(pqc)

            for qc in range(NQC):
                U = [psum.tile([D + 1, TQ], F32, tag="u", bufs=2, name=f"U{hh}")
                     for hh in range(HPC)]
                Us[qc] = U
                lag = first_lag if qc == 0 else lag_rest
                if fast_tail and qc == NQC - 1:
                    lag = 0
                for p in range(NKP):
                    exc = expp.tile([128, 2, 2 * TQ], vs_dt, tag="exp", name="exc")
                    for ko in range(2):
                        g = qc * NKT + 2 * p + ko
                        # scores one k-tile ahead: the next exp's input is in
                        # flight on the PE before any filler task can delay it
                        if g + 1 < n_iter:
                            emit_scores(g + 1)
                        ssc = sc_q.popleft()
                        # the -EXP_SHIFT cancels in the softmax normalize; it
                        # keeps tail scores under fp8e4's 240 max (no inf)
                        nc.scalar.activation(
                            out=exc[:, ko, :], in_=ssc, func=AF.Exp,
                            scale=exp_scale, bias=shift_col[:, 0:1],
                        )
                        pop_fillers()
                        it += 1
                        if len(pending) > lag:
                            pop_pending()

                    def make_av(qc=qc, p=p, exc=exc, U=U):
                        def emit_av():
                            for h in range(HPC):
                                if USE_FP8:
                                    nc.tensor.matmul(
                                        U[h],
                                        lhsT=VS[:, 2 * p:2 * p + 2, h * VSW:h * VSW + D + 1],
                                        rhs=exc[:, :, h * TQ:(h + 1) * TQ],
                                        start=(p == 0),
                                        stop=(p == NKP - 1),
                                        perf_mode=DR,
                                    )
                                else:
                                    for ko in range(2):
                                        nc.tensor.matmul(
                                            U[h],
                                            lhsT=VS[:, 2 * p + ko, h * VSW:h * VSW + D + 1],
                                            rhs=exc[:, ko, h * TQ:(h + 1) * TQ],
                                            start=(p == 0 and ko == 0),
                                            stop=(p == NKP - 1 and ko == 1),
                                        )
                        return emit_av
                    pending.append((qc, p, make_av()))
            while pending:
                pop_pending()

            while fill_q:
                fill_q.popleft()()
            return AO

        # ---- program ----
        # head: only the K chunk-0 chain + Q chunk-0 chain gate the first
        # exp; everything else is a (gated) demand-pulled PE filler task.
        nc.sync.dma_start(out=wt_q, in_=wq[:, :, :])
        load_x(0)
        for prm in (bq, bk, bv):
            bt = consts.tile([FPC, 1], F32, tag="bias", bufs=3)
            nc.sync.dma_start(out=bt, in_=prm[:, :])
            bias_ts.append(bt)
        nc.sync.dma_start(out=wt_v, in_=wv[:, :, :])
        nc.sync.dma_start(out=wo_t, in_=wo[:, :])
        load_x(1)
        q0 = qkv_tiles(0)
        QT0, KT0, VT0, VS0 = q0
        done = set()

        def tag_task(tag, fn):
            def run():
                fn()
                done.add(tag)
            return run

        qkv_chain(0, 1, KT0, 0)
        qkv_chain(0, 0, QT0, 0)
        done.add(("K", 0, 0))
        done.add(("Q", 0, 0))

        q1 = qkv_tiles(1)
        QT1, KT1, VT1, VS1 = q1
        fill0 = deque()
        for qc in range(1, NQC):
            fill0.append(tag_task(("K", 0, qc),
                                  lambda qc=qc: qkv_chain(0, 1, KT0, qc)))
        fill0.append(lambda: qkv_chain(0, 2, VT0, 0))
        for kt in range(4):
            fill0.append(tag_task(("vtr", 0, kt),
                                  lambda kt=kt: vtr_task(0, VT0, VS0, kt)))
        for qc in range(1, NQC):
            fill0.append(tag_task(("Q", 0, qc),
                                  lambda qc=qc: qkv_chain(0, 0, QT0, qc)))
            fill0.append(lambda qc=qc: qkv_chain(0, 2, VT0, qc))
            for kt in range(4 * qc, 4 * qc + 4):
                fill0.append(tag_task(("vtr", 0, kt),
                                      lambda kt=kt: vtr_task(0, VT0, VS0, kt)))
        # batch-1: K + Q0 + first half of V in attention(0)'s window;
        # the rest moves to attention(1)'s window for PE balance
        for qc in range(NQC):
            fill0.append(tag_task(("K", 1, qc),
                                  lambda qc=qc: qkv_chain(1, 1, KT1, qc)))
        fill0.append(tag_task(("Q", 1, 0), lambda: qkv_chain(1, 0, QT1, 0)))
        for qc in range(2):
            fill0.append(lambda qc=qc: qkv_chain(1, 2, VT1, qc))
            for kt in range(4 * qc, 4 * qc + 4):
                fill0.append(tag_task(("vtr", 1, kt),
                                      lambda kt=kt: vtr_task(1, VT1, VS1, kt)))
        fill0.append(tag_task(("Q", 1, 1), lambda: qkv_chain(1, 0, QT1, 1)))

        AO0 = attention(0, QT0, KT0, VS0, fill0, done, first_lag=4, lag_rest=3)

        fill1 = deque()
        for qc in range(2, NQC):
            fill1.append(lambda qc=qc: qkv_chain(1, 2, VT1, qc))
            for kt in range(4 * qc, 4 * qc + 4):
                fill1.append(tag_task(("vtr", 1, kt),
                                      lambda kt=kt: vtr_task(1, VT1, VS1, kt)))
        for qc in range(2, NQC):
            fill1.append(tag_task(("Q", 1, qc),
                                  lambda qc=qc: qkv_chain(1, 0, QT1, qc)))
        fill1.extend(proj_fillers(0, AO0, range(T // 128)))
        attention(1, QT1, KT1, VS1, fill1, done, self_proj=True,
                  fast_tail=True, first_lag=3, lag_rest=3)

    nc.finalize()
    return nc


_NC_CACHE = None


def _get_nc():
    global _NC_CACHE
    if _NC_CACHE is None:
        _NC_CACHE = build_nc()
    return _NC_CACHE


def _pair_w(w):
    """[C, FPC] -> [128, NET, FPC]: e-tile e at [:, e, :] (row ki of tile
    e is W row e*128+ki), so DoubleRow pair p contracts tiles 2p, 2p+1."""
    return np.ascontiguousarray(
        w.reshape(NET, 128, FPC).transpose(1, 0, 2)
    )


def make_in_maps(x, W_qkv, b_qkv, W_out):
    npdt = NPFP8 if USE_FP8 else NPBF16
    wq_s = WQ_SCALE if USE_FP8 else 1.0
    wv_s = WV_SCALE if USE_FP8 else 1.0
    xT = np.ascontiguousarray(x.transpose(0, 2, 1)).astype(npdt)
    Wob = W_out.astype(NPBF16)
    in_maps = []
    for c in range(NCORES):
        f0 = c * FPC
        in_maps.append(
            {
                "xT": xT,
                "wq": _pair_w(W_qkv[:, f0:f0 + FPC] * wq_s).astype(npdt),
                "wk": _pair_w(W_qkv[:, C + f0:C + f0 + FPC] * wq_s).astype(npdt),
                "wv": _pair_w(W_qkv[:, 2 * C + f0:2 * C + f0 + FPC] * wv_s).astype(npdt),
                "bq": np.ascontiguousarray(
                    b_qkv[f0:f0 + FPC, None] * wq_s).astype(np.float32),
                "bk": np.ascontiguousarray(
                    b_qkv[C + f0:C + f0 + FPC, None] * wq_s).astype(np.float32),
                "bv": np.ascontiguousarray(
                    b_qkv[2 * C + f0:2 * C + f0 + FPC, None] * wv_s).astype(np.float32),
                "wo": np.ascontiguousarray(Wob[f0:f0 + FPC, :]),
            }
        )
    return in_maps


def kernel(x, W_qkv, b_qkv, W_out, b_out, _trace=False, _trace_kwargs=None):
    x = np.asarray(x, dtype=np.float32)
    W_qkv = np.asarray(W_qkv, dtype=np.float32)
    b_qkv = np.asarray(b_qkv, dtype=np.float32)
    W_out = np.asarray(W_out, dtype=np.float32)
    b_out = np.asarray(b_out, dtype=np.float32)

    nc = _get_nc()
    in_maps = make_in_maps(x, W_qkv, b_qkv, W_out)
    res = run_bass_kernel_spmd(
        nc, in_maps, core_ids=list(range(NCORES)), trace=_trace,
        **(_trace_kwargs or {}),
    )
    y = res.results[0]["y"].astype(np.float64)
    for c in range(1, NCORES):
        y += res.results[c]["y"]
    y += b_out
    out = y.astype(np.float32)
    if _trace:
        return out, res
    return out
